# revision 39
# baseline (speedup 1.0000x reference)
"""Trainium2 Bass kernel for the 2-layer TransformerConv GNN (edge-parallel, 8 cores).

Strategy (edge parallel, per sharding hint):
  - Sort edges by dst; shard nodes into 8 equal slices of 1250; each core owns
    all edges whose dst falls in its slice, so segment-softmax and scatter-add
    are core-local (no softmax-stat collectives needed).
  - Layer-1 K/V node projections are computed replicated on every core
    (input x is replicated); layer-2 K/V are data-parallel over nodes followed
    by an AllGather of the fused K|V table.
  - Per core, edges are grouped into 64-node blocks; each block's <=9 tiles of
    128 edges accumulate their segment-sums in PSUM via a 0/1 selection-matrix
    matmul, so no indirect scatter is needed.
  - K|V rows are fetched per tile with indirect DMA (the only gather available
    on this runtime). Each block's edges are sorted by src so early tiles
    depend only on a prefix of the KV table - layer-1 gathers overlap the
    projection phase that writes the table.
  - Per tile: alpha = sum(qg*kg) + t[dst]*ea via one strided mul + one reduce
    (the t*ea product rides in column 128 of a [h,129] layout); ex=exp(alpha)
    is folded into V plus two extra columns [ex, ex*ea], so ONE matmul
    (sens^T @ [v*ex | ex | ex*ea]) yields the per-node message sums, softmax
    denominators, and edge-attr sums together:
       out_n = (sum_e ex_e*v_e + (sum_e ex_e*ea_e)*We) / (sum_e ex_e + 1e-16)
  - Q lives in SBUF ([64, NB, 258] per-block layout) - no DRAM round trip.
"""

import math

import numpy as np

N, E, H, C, D, F = 10000, 160000, 2, 128, 256, 128
R, NS, BLK = 8, 1250, 64
NB = (NS + BLK - 1) // BLK          # 20 blocks/core
NSPAD = NB * BLK                    # 1280
NFULL = 10112                       # 79*128, padded global node count
NT1 = NFULL // 128                  # 79
NTS = NSPAD // 128                  # 10
EPS = 1e-5
INV = 1.0 / math.sqrt(C)
G = 4                               # tiles per batched gather
NQ = 4                              # SWDGE queues (parallel Q7 descriptor gen)
LCH, NCH = 625, 2                   # KV2 AllGather chunk rows per core, chunks

_CACHE = {}


def _g2row(n):
    """Global node id -> row in the chunked-AllGather KV2 layout."""
    r, i = n // NS, n % NS
    return (i // LCH) * (R * LCH) + r * LCH + (i % LCH)


def _prepare(edge_index, edge_attr):
    """Host-side index preprocessing -> per-core tile arrays (data only)."""
    src = edge_index[0].astype(np.int64)
    dst = edge_index[1].astype(np.int64)
    ea = edge_attr[:, 0].astype(np.float32)
    perm = np.argsort(dst, kind="stable")
    sdst, ssrc, sea = dst[perm], src[perm], ea[perm]
    bounds = np.searchsorted(sdst, np.arange(0, N + 1, NS))
    core_info = []
    cnts = np.zeros((R, NB), dtype=np.int64)
    for r in range(R):
        lo, hi = bounds[r], bounds[r + 1]
        ldst = sdst[lo:hi] - NS * r
        bb = np.searchsorted(ldst, np.arange(0, NSPAD + 1, BLK))
        cnts[r] = np.diff(bb)
        core_info.append((lo, bb))
    n_b = np.maximum(1, np.ceil(cnts.max(axis=0) / 128).astype(np.int64))
    T = int(n_b.sum())
    g2 = _g2row(np.arange(N))
    import ml_dtypes
    bf16 = ml_dtypes.bfloat16
    per_core = []
    rdep = np.zeros((R, T), np.int64)
    for r in range(R):
        lo, bb = core_info[r]
        SRC = np.zeros((T, 128), np.int64)
        EAV = np.zeros((T, 2, 128), np.float32)
        EAV[:, 0] = 1.0
        S = np.zeros((T, 128, BLK), np.float32)
        t = 0
        for b in range(NB):
            base = lo + bb[b]
            nb_edges = bb[b + 1] - bb[b]
            # sort this block's edges by src so early tiles only touch the
            # low-row prefix of the KV table (gathers can start before the
            # whole table is written)
            bsl = slice(base, base + nb_edges)
            order = np.argsort(g2[ssrc[bsl]], kind="stable")
            bsrc = g2[ssrc[bsl]][order]
            bea = sea[bsl][order]
            bld = sdst[bsl][order] - NS * r
            for i in range(n_b[b]):
                e0 = min(128 * i, nb_edges)
                e1 = min(128 * (i + 1), nb_edges)
                cnt = e1 - e0
                if cnt > 0:
                    SRC[t, :cnt] = bsrc[e0:e1]   # g2row-space table rows
                    EAV[t, 1, :cnt] = bea[e0:e1]
                    S[t, np.arange(cnt), bld[e0:e1] - BLK * b] = 1.0
                    rdep[r, t] = int(bsrc[e1 - 1]) + 1
                t += 1
        # dma_gather int16 index layout: per batch of G tiles, flat index
        # i = j*128 + p (tile j, edge p) lives at [q*32 + i%16, base + i//16],
        # replicated at [q*32+16 + i%16] (the TX Q7 core reads its own copy).
        # q = k % NQ is the SWDGE queue the batch is issued on; queue q's
        # rx/tx Q7 core pair reads indices from partitions [q*32, q*32+32).
        nbat = (T + G - 1) // G
        idx16 = np.zeros((128, nbat * G * 8), np.int16)
        for k in range(nbat):
            t0 = k * G
            g_n = min(G, T - t0)
            q = k % NQ
            flat = np.zeros((G * 128,), np.int64)
            flat[:g_n * 128] = SRC[t0:t0 + g_n].reshape(-1)
            i = np.arange(G * 128)
            idx16[q * 32 + i % 16, k * (G * 8) + i // 16] = flat
            idx16[q * 32 + 16 + i % 16, k * (G * 8) + i // 16] = flat
        per_core.append(
            dict(
                IDX16=idx16,                                      # [128, nbat*G*8]
                # [128, 2T]: col 2t = 1.0, col 2t+1 = edge_attr value
                EAV=np.ascontiguousarray(EAV.transpose(2, 0, 1).reshape(128, 2 * T)).astype(bf16),
                SEN=np.ascontiguousarray(
                    S.transpose(1, 0, 2).reshape(128, T * BLK)).astype(bf16),
                SNE=np.ascontiguousarray(
                    S.transpose(2, 0, 1).reshape(BLK, T * 128)).astype(bf16),
            )
        )
    rdep_max = rdep.max(axis=0)
    return per_core, n_b, T, rdep_max


def _build(T, n_b, rdep=None, zero_bias=False, stage=99):
    """Build + schedule the (shared-across-cores) Bass program.

    stage: 1=projections only, 2=+edge1, 3=+node1, 4=+proj2/AllGather,
           5=+edge2, 99=full.
    """
    import concourse.bass as bass
    import concourse.mybir as mybir
    import concourse.tile as tile
    from concourse import bacc
    from concourse.masks import make_identity

    f32 = mybir.dt.float32
    bf16 = mybir.dt.bfloat16
    i16 = mybir.dt.int16
    Alu = mybir.AluOpType
    Act = mybir.ActivationFunctionType
    AX = mybir.AxisListType

    nc = bacc.Bacc("TRN2", target_bir_lowering=False, debug=False,
                   enable_asserts=False, num_devices=R, num_swdge_queues=NQ)

    def din(name, shape, dtype=f32):
        return nc.dram_tensor(name, shape, dtype, kind="ExternalInput").ap()

    NBAT = (T + G - 1) // G
    XTF = din("XTF", [128, NFULL], bf16)
    XTS = din("XTS", [128, NSPAD], bf16)
    IDX16 = din("IDX16", [128, NBAT * G * 8], i16)
    EAV = din("EAV", [128, 2 * T], bf16)
    SEN = din("SEN", [128, T * BLK], bf16)
    SNE = din("SNE", [BLK, T * 128], bf16)
    W1 = {k: din(k, [128, 256], bf16) for k in ("WK1", "WV1", "WQ1", "WS1")}
    B1 = {k: din(k, [128, 256]) for k in ("BK1", "BV1", "BQ1", "BS1")}
    WE1 = din("WE1", [128, 256])
    W2 = {k: din(k, [128, 512], bf16) for k in ("WK2", "WV2", "WQ2", "WS2")}
    B2 = {k: din(k, [128, 256]) for k in ("BK2", "BV2", "BQ2", "BS2")}
    WE2 = din("WE2", [128, 256])
    G1T, B1TT = din("G1T", [128, 2]), din("B1TT", [128, 2])
    G2T, B2TT = din("G2T", [128, 2]), din("B2TT", [128, 2])
    OUTT = nc.dram_tensor("OUTT", [256, NSPAD], f32, kind="ExternalOutput").ap()

    from contextlib import ExitStack

    with tile.TileContext(nc) as tc, ExitStack() as ctx:
        cp = ctx.enter_context(tc.tile_pool(name="const", bufs=1))
        dram = ctx.enter_context(tc.tile_pool(name="dram", bufs=1, space="DRAM"))
        lhsp = ctx.enter_context(tc.tile_pool(name="lhsp", bufs=2))
        kvtp = ctx.enter_context(tc.tile_pool(name="kvtp", bufs=2))
        kvt2p = ctx.enter_context(tc.tile_pool(name="kvt2p", bufs=2))
        qsp = ctx.enter_context(tc.tile_pool(name="qsp", bufs=1))
        kvgp = ctx.enter_context(tc.tile_pool(name="kvgp", bufs=4))
        vvp = ctx.enter_context(tc.tile_pool(name="vvp", bufs=8))
        scrp = ctx.enter_context(tc.tile_pool(name="scrp", bufs=8))
        smp = ctx.enter_context(tc.tile_pool(name="smp", bufs=16))
        msop = ctx.enter_context(tc.tile_pool(name="msop", bufs=4))
        hop = ctx.enter_context(tc.tile_pool(name="hop", bufs=4))
        htp = ctx.enter_context(tc.tile_pool(name="htp", bufs=1))
        atp = ctx.enter_context(tc.tile_pool(name="atp", bufs=1))
        bigp = ctx.enter_context(tc.tile_pool(name="bigp", bufs=1))
        gpp = ctx.enter_context(tc.tile_pool(name="gpp", bufs=2, space="PSUM"))
        qgpp = ctx.enter_context(tc.tile_pool(name="qgpp", bufs=2, space="PSUM"))
        segp = ctx.enter_context(tc.tile_pool(name="segp", bufs=2, space="PSUM"))
        segdp = ctx.enter_context(tc.tile_pool(name="segdp", bufs=2, space="PSUM"))

        def pj_tile():
            return gpp.tile([128, 512], f32, name="pj", tag="gp")

        def evict(out_ap, ps_ap, bias_tile):
            if zero_bias:
                nc.any.tensor_copy(out=out_ap, in_=ps_ap)
            else:
                nc.any.tensor_tensor(out=out_ap, in0=ps_ap, in1=bias_tile,
                                     op=Alu.add)

        # ---- DRAM scratch ----
        KV1 = dram.tile([NFULL, 512], bf16, name="KV1")
        MS1 = dram.tile([2, NSPAD, 130], f32, name="MS1")
        KV2S = dram.tile([NS, 512], bf16, name="KV2S")
        KV2F = dram.tile([N, 512], bf16, name="KV2F")
        MS2 = dram.tile([2, NSPAD, 130], f32, name="MS2")
        STI1 = dram.tile([128, 4], f32, name="STI1")
        STO1 = dram.tile([128 * R, 4], f32, name="STO1", addr_space="Shared")
        STI2 = dram.tile([128, 4], f32, name="STI2")
        STO2 = dram.tile([128 * R, 4], f32, name="STO2", addr_space="Shared")

        # ---- resident SBUF constants ----
        def load(name, ap, shape, dtype=f32):
            t = cp.tile(shape, dtype, name=name)
            nc.sync.dma_start(t[:], ap[:, :])
            return t

        xts = load("xts", XTS, [128, NSPAD], bf16)
        w1 = {k: load(k.lower(), v, [128, 256], bf16) for k, v in W1.items()}
        we1 = load("we1", WE1, [128, 256])
        # edge-phase constants loaded EARLY so layer-1 gathers can start as
        # soon as the first KV rows land (they sit on the same sync DMA queue)
        idx16s = load("idx16s", IDX16, [128, NBAT * G * 8], i16)
        eavs = load("eavs", EAV, [128, 2 * T], bf16)
        w2 = {k: load(k.lower(), v, [128, 512], bf16) for k, v in W2.items()}
        if zero_bias:
            b1 = {k: None for k in B1}
            b2 = {k: None for k in B2}
        else:
            b1 = {k: load(k.lower(), v, [128, 256]) for k, v in B1.items()}
            b2 = {k: load(k.lower(), v, [128, 256]) for k, v in B2.items()}
        we2 = load("we2", WE2, [128, 256])
        g1t = load("g1t", G1T, [128, 2])
        b1tt = load("b1tt", B1TT, [128, 2])
        g2t = load("g2t", G2T, [128, 2])
        b2tt = load("b2tt", B2TT, [128, 2])
        ident = cp.tile([128, 128], f32, name="ident")
        make_identity(nc, ident[:])

        ht = [htp.tile([128, NSPAD], f32, name=f"ht{h}") for h in range(2)]

        # ================= q projections (per 64-node block, Q stays in SBUF)
        def q_proj(lhs_chunks, wq, bq, we, lname):
            qs = qsp.tile([BLK, NB, 264], bf16, name="qs")
            nkc = len(lhs_chunks)
            for b in range(NB):
                sl = slice(b * BLK, (b + 1) * BLK)
                ps = pj_tile()
                for kc in range(nkc):
                    nc.tensor.matmul(ps[0:BLK, 0:256], lhsT=lhs_chunks[kc][:, sl],
                                     rhs=wq[:, kc * 256:(kc + 1) * 256],
                                     start=(kc == 0), stop=(kc == nkc - 1))
                if zero_bias:
                    nc.any.tensor_copy(out=qs[:, b, 0:256], in_=ps[0:BLK, 0:256])
                else:
                    nc.any.tensor_tensor(out=qs[:, b, 0:256],
                                         in0=ps[0:BLK, 0:256],
                                         in1=bq[0:BLK, :], op=Alu.add)
                scr = scrp.tile([128, 256], bf16, name="scr")
                nc.vector.tensor_mul(out=scr[0:BLK, :], in0=ps[0:BLK, 0:256],
                                     in1=we[0:BLK, 0:256])
                tsum = smp.tile([BLK, 2], f32, name="tsum")
                nc.vector.reduce_sum(
                    out=tsum[:],
                    in_=scr[0:BLK, :].rearrange("p (h c) -> p h c", c=128),
                    axis=AX.X)
                nc.any.tensor_copy(out=qs[:, b, 256:258], in_=tsum[:])
            return qs

        # q first: edge-phase compute for block b only needs qs[b], while the
        # K/V table fills behind it (src-sorted tiles gate on row prefixes).
        qs1 = q_proj([xts], w1["WQ1"], b1["BQ1"], we1, "qs1")

        # ================= layer-1 K/V projections =================
        # K/V for ALL nodes, replicated on every core.
        nt = 0
        lhs = None
        while nt < NT1:
            if nt % 8 == 0:
                nl = min(8, NT1 - nt)
                lhs = lhsp.tile([128, 8 * 128], bf16, name="lhs")
                nc.sync.dma_start(lhs[:, 0:nl * 128],
                                  XTF[:, nt * 128:(nt + nl) * 128])
                lbase = nt
            nj = min(4, NT1 - nt)
            kvt = kvtp.tile([128, 4 * 512], bf16, name="kvt")
            for j in range(nj):
                jj = nt - lbase + j
                ps = pj_tile()
                for Wn, off in (("WK1", 0), ("WV1", 256)):
                    nc.tensor.matmul(ps[:, off:off + 256],
                                     lhsT=lhs[:, jj * 128:(jj + 1) * 128],
                                     rhs=w1[Wn][:], start=True, stop=True)
                if zero_bias:
                    # split the eviction across Scalar and Vector so the
                    # PSUM->SBUF copies run in parallel with the matmuls
                    nc.scalar.copy(out=kvt[:, j * 512:j * 512 + 256],
                                   in_=ps[:, 0:256])
                    nc.vector.tensor_copy(
                        out=kvt[:, j * 512 + 256:(j + 1) * 512],
                        in_=ps[:, 256:512])
                else:
                    for Bn, off in (("BK1", 0), ("BV1", 256)):
                        evict(kvt[:, j * 512 + off:j * 512 + off + 256],
                              ps[:, off:off + 256], b1[Bn][:])
            nc.scalar.dma_start(
                KV1[nt * 128:(nt + nj) * 128, :].rearrange("(j p) c -> p j c", p=128),
                kvt[:, 0:nj * 512].rearrange("p (j c) -> p j c", c=512))
            nt += nj

        # selection matrices: needed only once edge compute starts, so load
        # them behind the projection operand traffic on the sync DMA queue
        sens = load("sens", SEN, [128, T * BLK], bf16)
        snes = load("snes", SNE, [BLK, T * 128], bf16)

        # ================= edge phase =================
        def edge_phase(KVt, idxs, qs, MSt, tile_rdep=None, round_to=512,
                       on_block=None):
            kv_rows = KVt.shape[0]
            # tile -> (block, is_first_in_block, is_last_in_block)
            tinfo = []
            for b in range(NB):
                nb = int(n_b[b])
                for i in range(nb):
                    tinfo.append((b, i == 0, i == nb - 1))
            assert len(tinfo) == T
            state = {"msv": None}

            def phase1(t0, g_n):
                """Gather + alpha for a batch -> (kvg, exfs)."""
                if tile_rdep is None:
                    rows = kv_rows
                else:
                    rmax = int(max(tile_rdep[t0:t0 + g_n]))
                    rows = min(kv_rows,
                               ((rmax + round_to - 1) // round_to) * round_to)
                k = t0 // G
                kvg = kvgp.tile([128, G, 512], bf16, name="kvg")
                nc.gpsimd.dma_gather(
                    out_ap=kvg[:, 0:g_n, :], in_ap=KVt[0:rows, :],
                    idxs_ap=idxs[:, k * (G * 8):k * (G * 8) + g_n * 8],
                    num_idxs=g_n * 128, num_idxs_reg=g_n * 128,
                    elem_size=512, queue_num=k % NQ)
                eats = smp.tile([128, G, 2], f32, name="eats")
                for g in range(g_n):
                    t = t0 + g
                    b = tinfo[t][0]
                    qgps = qgpp.tile([128, 258], f32, name="qgps", tag="qg")
                    nc.tensor.matmul(out=qgps[:],
                                     lhsT=snes[:, t * 128:(t + 1) * 128],
                                     rhs=qs[:, b, 0:258], start=True, stop=True)
                    # scr layout [p, h, 129]: cols 0:128 = qg*kg, col 128 =
                    # t[dst]*ea -> one reduce yields eat directly.
                    scr = scrp.tile([128, 258], bf16, name="scr")
                    scr3 = scr[:].rearrange("p (h c) -> p h c", c=129)
                    nc.vector.tensor_mul(
                        out=scr3[:, :, 0:128],
                        in0=qgps[:, 0:256].rearrange("p (h c) -> p h c", c=128),
                        in1=kvg[:, g, 0:256].rearrange("p (h c) -> p h c", c=128))
                    nc.vector.tensor_mul(
                        out=scr3[:, :, 128:129],
                        in0=qgps[:, 256:258].rearrange("p (h c) -> p h c", c=1),
                        in1=eavs[:, 2 * t + 1:2 * t + 2].to_broadcast([128, 2, 1]))
                    nc.vector.reduce_sum(out=eats[:, g, :], in_=scr3, axis=AX.X)
                exfs = smp.tile([128, G, 2], f32, name="exfs")
                nc.scalar.activation(out=exfs[:, 0:g_n, :],
                                     in_=eats[:, 0:g_n, :], func=Act.Exp)
                return kvg, exfs

            def phase2(t0, g_n, kvg, exfs):
                """exp-scaled selection + segment-sum matmuls for a batch.

                sens_ex[e, (h,n)] = sens[e, n] * exp(alpha)[e, h]; the V
                aggregation, softmax denominators and ea-sums then come out
                of two matmuls with rhs = raw gathered V and [1|ea] columns:
                  msv[(h,n), 0:256]   += sens_ex^T @ v      (cols h*128.. valid)
                  msv[(h,n), 256:258] += sens_ex^T @ [1|ea] (den, eas)
                """
                for g in range(g_n):
                    t = t0 + g
                    b, first, last = tinfo[t]
                    sex = vvp.tile([128, 128], bf16, name="sex")
                    nc.vector.tensor_mul(
                        out=sex[:].rearrange("p (h n) -> p h n", n=BLK),
                        in0=sens[:, t * BLK:(t + 1) * BLK].unsqueeze(1)
                            .to_broadcast([128, 2, BLK]),
                        in1=exfs[:, g, :].unsqueeze(2).to_broadcast([128, 2, BLK]))
                    if first:
                        # separate PSUM tiles: interleaved start/stop groups
                        # sharing one PSUM tile accumulate incorrectly on HW
                        state["msv"] = segp.tile([128, 256], f32, name="msv")
                        state["msd"] = segdp.tile([128, 2], f32, name="msd")
                    nc.tensor.matmul(out=state["msv"][:],
                                     lhsT=sex[:], rhs=kvg[:, g, 256:512],
                                     start=first, stop=last)
                    nc.tensor.matmul(out=state["msd"][:],
                                     lhsT=sex[:], rhs=eavs[:, 2 * t:2 * t + 2],
                                     start=first, stop=last)
                    if last:
                        for h in range(2):
                            moh = msop.tile([BLK, 130], f32, name="moh")
                            nc.any.tensor_copy(
                                out=moh[:, 0:128],
                                in_=state["msv"][h * 64:(h + 1) * 64,
                                                 h * 128:(h + 1) * 128])
                            nc.any.tensor_copy(
                                out=moh[:, 128:130],
                                in_=state["msd"][h * 64:(h + 1) * 64, :])
                            nc.scalar.dma_start(
                                MSt[h, b * BLK:(b + 1) * BLK, :], moh[:])
                        if on_block is not None:
                            on_block(b)

            # software-pipelined emission: phase1(k+1) is enqueued before
            # phase2(k) so the vector queue never stalls on exp(k)
            batches = [(t0, min(G, T - t0)) for t0 in range(0, T, G)]
            prev = None
            for t0, g_n in batches:
                cur = (t0, g_n) + phase1(t0, g_n)
                if prev is not None:
                    phase2(*prev)
                prev = cur
            phase2(*prev)

        def bail():
            z = bigp.tile([128, NSPAD], f32, name="sq")
            nc.vector.memset(z[:], 0.0)
            for half in range(2):
                nc.sync.dma_start(OUTT[half * 128:(half + 1) * 128, :], z[:])

        # ================= node phase =================
        def node_tile(mt, MSt, lhs_chunks, ws, bs, we):
                nkc = len(lhs_chunks)
                sl = slice(mt * 128, (mt + 1) * 128)
                # per-head message tiles for this 128-node slice (2 blocks):
                # cols 0:128 = msg_h, col 128 = den_h, col 129 = eas_h
                msh = []
                for h in range(2):
                    m = msop.tile([128, 130], f32, name=f"msi{h}")
                    nc.sync.dma_start(m[:], MSt[h, sl, :])
                    msh.append(m)
                ps = pj_tile()
                for kc in range(nkc):
                    nc.tensor.matmul(ps[:, 0:256], lhsT=lhs_chunks[kc][:, sl],
                                     rhs=ws[:, kc * 256:(kc + 1) * 256],
                                     start=(kc == 0), stop=(kc == nkc - 1))
                for h in range(2):
                    m = msh[h]
                    den = smp.tile([128, 1], f32, name="den")
                    nc.vector.tensor_scalar_add(out=den[:], in0=m[:, 128:129],
                                                scalar1=1e-16)
                    rec = smp.tile([128, 1], f32, name="rec")
                    nc.vector.reciprocal(out=rec[:], in_=den[:])
                    tmp = hop.tile([128, 128], f32, name="tmpn")
                    # tmp = we_h * eas_h + msg_h
                    nc.vector.scalar_tensor_tensor(
                        out=tmp[:], in0=we[:, h * 128:(h + 1) * 128],
                        scalar=m[:, 129:130], in1=m[:, 0:128],
                        op0=Alu.mult, op1=Alu.add)
                    ho = hop.tile([128, 128], f32, name="ho")
                    # ho = tmp / den + skip-projection
                    nc.vector.scalar_tensor_tensor(
                        out=ho[:], in0=tmp[:], scalar=rec[:],
                        in1=ps[:, h * 128:(h + 1) * 128],
                        op0=Alu.mult, op1=Alu.add)
                    if not zero_bias:
                        nc.vector.tensor_add(out=ho[:], in0=ho[:],
                                             in1=bs[:, h * 128:(h + 1) * 128])
                    tp = gpp.tile([128, 512], f32, name="pj", tag="gp")
                    nc.tensor.transpose(out=tp[:, 0:128], in_=ho[:],
                                        identity=ident[:])
                    nc.vector.tensor_copy(out=ht[h][:, sl], in_=tp[:, 0:128])

        def node_finale(gt, bt, at_names, STI, STO, at_dt):
            stt = smp.tile([128, 4], f32, name="stt")
            for half in range(2):
                nc.vector.reduce_sum(out=stt[:, half:half + 1],
                                     in_=ht[half][:, 0:NS], axis=AX.X)
                sq = bigp.tile([128, NSPAD], f32, name="sq")
                nc.scalar.activation(out=sq[:, 0:NS], in_=ht[half][:, 0:NS],
                                     func=Act.Square,
                                     accum_out=stt[:, 2 + half:3 + half])
            nc.sync.dma_start(STI[:, :], stt[:])
            nc.gpsimd.collective_compute(
                "AllGather", Alu.bypass, replica_groups=[list(range(R))],
                ins=[STI[:].opt()], outs=[STO[:].opt()])
            gsta = smp.tile([128, 4, R], f32, name="gsta")
            nc.sync.dma_start(
                gsta[:], STO[:].rearrange("(r p) k -> p k r", p=128))
            gst = smp.tile([128, 4], f32, name="gst")
            nc.vector.reduce_sum(out=gst[:], in_=gsta[:], axis=AX.X)
            mean = smp.tile([128, 2], f32, name="mean")
            nc.vector.tensor_scalar_mul(out=mean[:], in0=gst[:, 0:2],
                                        scalar1=1.0 / N)
            var = smp.tile([128, 2], f32, name="var")
            nc.vector.tensor_scalar_mul(out=var[:], in0=gst[:, 2:4],
                                        scalar1=1.0 / N)
            m2 = smp.tile([128, 2], f32, name="m2")
            nc.vector.tensor_mul(out=m2[:], in0=mean[:], in1=mean[:])
            nc.vector.tensor_sub(out=var[:], in0=var[:], in1=m2[:])
            nc.vector.tensor_scalar_add(out=var[:], in0=var[:], scalar1=EPS)
            sd = smp.tile([128, 2], f32, name="sd")
            nc.scalar.activation(out=sd[:], in_=var[:], func=Act.Sqrt)
            rsd = smp.tile([128, 2], f32, name="rsd")
            nc.vector.reciprocal(out=rsd[:], in_=sd[:])
            sc2 = smp.tile([128, 2], f32, name="sc2")
            nc.vector.tensor_mul(out=sc2[:], in0=gt[:], in1=rsd[:])
            sh2 = smp.tile([128, 2], f32, name="sh2")
            nc.vector.tensor_mul(out=sh2[:], in0=mean[:], in1=sc2[:])
            nc.vector.tensor_sub(out=sh2[:], in0=bt[:], in1=sh2[:])
            at = [atp.tile([128, NSPAD], at_dt, name=nm) for nm in at_names]
            for half in range(2):
                nc.scalar.activation(out=at[half][:], in_=ht[half][:],
                                     func=Act.Relu,
                                     scale=sc2[:, half:half + 1],
                                     bias=sh2[:, half:half + 1])
            return at

        if stage >= 2:
            def cb1(b):
                # emit node-tile work as soon as its two blocks' messages
                # are stored; it executes in edge-phase engine slack
                if b % 2 == 1 and stage >= 3:
                    node_tile(b // 2, MS1, [xts], w1["WS1"], b1["BS1"], we1)
            edge_phase(KV1, idx16s, qs1, MS1, tile_rdep=rdep, on_block=cb1)
        if stage >= 3:
            at1 = node_finale(g1t, b1tt, ["at10", "at11"], STI1, STO1, bf16)

        # ===== layer-2 K/V projections (data-parallel + chunked AllGather) ===
        if stage >= 4:
          for mt in range(NTS):
            sl = slice(mt * 128, (mt + 1) * 128)
            kvt = kvt2p.tile([128, 512], bf16, name="kvt2")
            ps = pj_tile()
            for Wn, off in (("WK2", 0), ("WV2", 256)):
                for kc in range(2):
                    nc.tensor.matmul(ps[:, off:off + 256], lhsT=at1[kc][:, sl],
                                     rhs=w2[Wn][:, kc * 256:(kc + 1) * 256],
                                     start=(kc == 0), stop=(kc == 1))
            if zero_bias:
                nc.any.tensor_copy(out=kvt[:], in_=ps[:])
            else:
                for Bn, off in (("BK2", 0), ("BV2", 256)):
                    evict(kvt[:, off:off + 256], ps[:, off:off + 256],
                          b2[Bn][:])
            if mt < NTS - 1:
                nc.scalar.dma_start(KV2S[sl, :], kvt[:])
            else:
                nc.scalar.dma_start(KV2S[mt * 128:NS, :],
                                  kvt[:NS - mt * 128, :])
          for j in range(NCH):
              nc.gpsimd.collective_compute(
                  "AllGather", Alu.bypass, replica_groups=[list(range(R))],
                  ins=[KV2S[j * LCH:(j + 1) * LCH, :].opt()],
                  outs=[KV2F[j * R * LCH:(j + 1) * R * LCH, :].opt()])
          qs2 = q_proj(at1, w2["WQ2"], b2["BQ2"], we2, "qs2")

        if stage >= 5:
            def cb2(b):
                if b % 2 == 1 and stage >= 99:
                    node_tile(b // 2, MS2, at1, w2["WS2"], b2["BS2"], we2)
            edge_phase(KV2F, idx16s, qs2, MS2, tile_rdep=rdep,
                       round_to=R * LCH, on_block=cb2)

        if stage >= 99:
            at2 = node_finale(g2t, b2tt, ["at20", "at21"], STI2, STO2, f32)
            for half in range(2):
                nc.sync.dma_start(OUTT[half * 128:(half + 1) * 128, :],
                                  at2[half][:])
        else:
            bail()

    nc.compile()
    return nc


def _pack_inputs(inputs, per_core):
    import ml_dtypes
    bfh = ml_dtypes.bfloat16
    x = np.asarray(inputs["x"], np.float32)
    # XTF columns in g2row order so the KV1 table (written sequentially by
    # the projection loop) lands in the same row space the layer-2
    # chunked-AllGather table uses; src indices are g2row for both layers
    xT = np.zeros((128, NFULL), np.float32)
    xT[:, _g2row(np.arange(N))] = x.T
    common = {"XTF": np.ascontiguousarray(xT).astype(bfh)}

    def bc(v):
        return np.ascontiguousarray(np.broadcast_to(
            np.asarray(v, np.float32).reshape(1, -1), (128, v.shape[-1])))

    def chunks2(w):
        w = np.asarray(w, np.float32)
        return np.ascontiguousarray(np.concatenate([w[0:128], w[128:256]], axis=1))

    for L, nm in ((1, "1"), (2, "2")):
        for key, wn in (("Wk", "WK"), ("Wv", "WV"), ("Wq", "WQ"), ("Ws", "WS")):
            w = np.asarray(inputs[key + nm], np.float32)
            if wn == "WQ":
                w = w * np.float32(INV)
            if L == 1:
                common[wn + nm] = np.ascontiguousarray(w).astype(bfh)
            else:
                common[wn + nm] = chunks2(w).astype(bfh)
        for key, bn in (("bk", "BK"), ("bv", "BV"), ("bq", "BQ"), ("bs", "BS")):
            b = np.asarray(inputs[key + nm], np.float32)
            if bn == "BQ":
                b = b * np.float32(INV)
            common[bn + nm] = bc(b)
        common["WE" + nm] = bc(np.asarray(inputs["We" + nm], np.float32).reshape(-1))
        common["G" + nm + "T"] = np.ascontiguousarray(
            np.asarray(inputs["g" + nm], np.float32).reshape(2, 128).T)
        common["B" + nm + "TT"] = np.ascontiguousarray(
            np.asarray(inputs["b" + nm], np.float32).reshape(2, 128).T)

    in_maps = []
    for r in range(R):
        m = dict(common)
        xs = np.zeros((128, NSPAD), np.float32)
        xs[:, :NS] = x[NS * r:NS * (r + 1)].T
        m["XTS"] = np.ascontiguousarray(xs).astype(bfh)
        m.update(per_core[r])
        in_maps.append(m)
    return in_maps


def kernel(**inputs):
    from concourse import bass_utils

    edge_index = np.asarray(inputs["edge_index"])
    edge_attr = np.asarray(inputs["edge_attr"], np.float32)
    zb = all(not np.any(np.asarray(inputs[k]))
             for k in ("bq1", "bk1", "bv1", "bs1", "bq2", "bk2", "bv2", "bs2"))
    key = (hash(edge_index.tobytes()), zb)
    if key not in _CACHE:
        per_core, n_b, T, rdep = _prepare(edge_index, edge_attr)
        nc = _build(T, n_b, rdep=rdep, zero_bias=zb)
        _CACHE[key] = (nc, per_core)
    nc, per_core = _CACHE[key]

    in_maps = _pack_inputs(inputs, per_core)
    import os
    trace = bool(int(os.environ.get("KBENCH_TRACE", "0")))
    res = bass_utils.run_bass_kernel_spmd(
        nc, in_maps, core_ids=list(range(R)), trace=trace)
    kernel.last_result = res
    out = np.concatenate(
        [res.results[r]["OUTT"][:, :NS].T for r in range(R)], axis=0)
    return np.ascontiguousarray(out)



# revision 40
# speedup vs baseline: 1.1682x; 1.1682x over previous
"""Trainium2 Bass kernel for the 2-layer TransformerConv GNN (edge-parallel, 8 cores).

Strategy (edge parallel, per sharding hint):
  - Sort edges by dst; shard nodes into 8 equal slices of 1250; each core owns
    all edges whose dst falls in its slice, so segment-softmax and scatter-add
    are core-local (no softmax-stat collectives needed).
  - Layer-1 K/V node projections are computed replicated on every core
    (input x is replicated); layer-2 K/V are data-parallel over nodes followed
    by an AllGather of the fused K|V table.
  - Per core, edges are grouped into 64-node blocks; each block's <=9 tiles of
    128 edges accumulate their segment-sums in PSUM via a 0/1 selection-matrix
    matmul, so no indirect scatter is needed.
  - K|V rows are fetched per tile with indirect DMA (the only gather available
    on this runtime). Each block's edges are sorted by src so early tiles
    depend only on a prefix of the KV table - layer-1 gathers overlap the
    projection phase that writes the table.
  - Per tile: alpha = sum(qg*kg) + t[dst]*ea via one strided mul + one reduce
    (the t*ea product rides in column 128 of a [h,129] layout); ex=exp(alpha)
    is folded into V plus two extra columns [ex, ex*ea], so ONE matmul
    (sens^T @ [v*ex | ex | ex*ea]) yields the per-node message sums, softmax
    denominators, and edge-attr sums together:
       out_n = (sum_e ex_e*v_e + (sum_e ex_e*ea_e)*We) / (sum_e ex_e + 1e-16)
  - Q lives in SBUF ([64, NB, 258] per-block layout) - no DRAM round trip.
"""

import math

import numpy as np

N, E, H, C, D, F = 10000, 160000, 2, 128, 256, 128
R, NS, BLK = 8, 1250, 64
NB = (NS + BLK - 1) // BLK          # 20 blocks/core
NSPAD = NB * BLK                    # 1280
NFULL = 10112                       # 79*128, padded global node count
NT1 = NFULL // 128                  # 79
NTS = NSPAD // 128                  # 10
EPS = 1e-5
INV = 1.0 / math.sqrt(C)
G = 4                               # tiles per batched gather
NQ = 4                              # SWDGE queues (parallel Q7 descriptor gen)
LCH, NCH = 625, 2                   # KV2 AllGather chunk rows per core, chunks

_CACHE = {}


def _g2row(n):
    """Global node id -> row in the chunked-AllGather KV2 layout."""
    r, i = n // NS, n % NS
    return (i // LCH) * (R * LCH) + r * LCH + (i % LCH)


def _prepare(edge_index, edge_attr):
    """Host-side index preprocessing -> per-core tile arrays (data only)."""
    src = edge_index[0].astype(np.int64)
    dst = edge_index[1].astype(np.int64)
    ea = edge_attr[:, 0].astype(np.float32)
    perm = np.argsort(dst, kind="stable")
    sdst, ssrc, sea = dst[perm], src[perm], ea[perm]
    bounds = np.searchsorted(sdst, np.arange(0, N + 1, NS))
    core_info = []
    cnts = np.zeros((R, NB), dtype=np.int64)
    for r in range(R):
        lo, hi = bounds[r], bounds[r + 1]
        ldst = sdst[lo:hi] - NS * r
        bb = np.searchsorted(ldst, np.arange(0, NSPAD + 1, BLK))
        cnts[r] = np.diff(bb)
        core_info.append((lo, bb))
    n_b = np.maximum(1, np.ceil(cnts.max(axis=0) / 128).astype(np.int64))
    T = int(n_b.sum())
    g2 = _g2row(np.arange(N))
    import ml_dtypes
    bf16 = ml_dtypes.bfloat16
    per_core = []
    rdep = np.zeros((R, T), np.int64)
    for r in range(R):
        lo, bb = core_info[r]
        SRC = np.zeros((T, 128), np.int64)
        EAV = np.zeros((T, 2, 128), np.float32)
        EAV[:, 0] = 1.0
        S = np.zeros((T, 128, BLK), np.float32)
        t = 0
        for b in range(NB):
            base = lo + bb[b]
            nb_edges = bb[b + 1] - bb[b]
            # sort this block's edges by src so early tiles only touch the
            # low-row prefix of the KV table (gathers can start before the
            # whole table is written)
            bsl = slice(base, base + nb_edges)
            order = np.argsort(ssrc[bsl], kind="stable")
            bsrc = ssrc[bsl][order]
            bea = sea[bsl][order]
            bld = sdst[bsl][order] - NS * r
            for i in range(n_b[b]):
                e0 = min(128 * i, nb_edges)
                e1 = min(128 * (i + 1), nb_edges)
                cnt = e1 - e0
                if cnt > 0:
                    SRC[t, :cnt] = bsrc[e0:e1]   # g2row-space table rows
                    EAV[t, 1, :cnt] = bea[e0:e1]
                    S[t, np.arange(cnt), bld[e0:e1] - BLK * b] = 1.0
                    rdep[r, t] = int(bsrc[e1 - 1]) + 1
                t += 1
        # dma_gather int16 index layout: per batch of G tiles, flat index
        # i = j*128 + p (tile j, edge p) lives at [q*32 + i%16, base + i//16],
        # replicated at [q*32+16 + i%16] (the TX Q7 core reads its own copy).
        # q = k % NQ is the SWDGE queue the batch is issued on; queue q's
        # rx/tx Q7 core pair reads indices from partitions [q*32, q*32+32).
        nbat = (T + G - 1) // G
        idx16 = np.zeros((128, nbat * G * 8), np.int16)
        for k in range(nbat):
            t0 = k * G
            g_n = min(G, T - t0)
            q = k % NQ
            flat = np.zeros((G * 128,), np.int64)
            flat[:g_n * 128] = SRC[t0:t0 + g_n].reshape(-1)
            i = np.arange(G * 128)
            idx16[q * 32 + i % 16, k * (G * 8) + i // 16] = flat
            idx16[q * 32 + 16 + i % 16, k * (G * 8) + i // 16] = flat
        per_core.append(
            dict(
                IDX16=idx16,                                      # [128, nbat*G*8]
                # [128, 2T]: col 2t = 1.0, col 2t+1 = edge_attr value
                EAV=np.ascontiguousarray(EAV.transpose(2, 0, 1).reshape(128, 2 * T)).astype(bf16),
                SEN=np.ascontiguousarray(
                    S.transpose(1, 0, 2).reshape(128, T * BLK)).astype(bf16),
                SNE=np.ascontiguousarray(
                    S.transpose(2, 0, 1).reshape(BLK, T * 128)).astype(bf16),
            )
        )
    rdep_max = rdep.max(axis=0)
    return per_core, n_b, T, rdep_max


def _build(T, n_b, rdep=None, zero_bias=False, stage=99):
    """Build + schedule the (shared-across-cores) Bass program.

    stage: 1=projections only, 2=+edge1, 3=+node1, 4=+proj2/AllGather,
           5=+edge2, 99=full.
    """
    import concourse.bass as bass
    import concourse.mybir as mybir
    import concourse.tile as tile
    from concourse import bacc
    from concourse.masks import make_identity

    f32 = mybir.dt.float32
    bf16 = mybir.dt.bfloat16
    i16 = mybir.dt.int16
    Alu = mybir.AluOpType
    Act = mybir.ActivationFunctionType
    AX = mybir.AxisListType

    nc = bacc.Bacc("TRN2", target_bir_lowering=False, debug=False,
                   enable_asserts=False, num_devices=R, num_swdge_queues=NQ)

    def din(name, shape, dtype=f32):
        return nc.dram_tensor(name, shape, dtype, kind="ExternalInput").ap()

    NBAT = (T + G - 1) // G
    XTF = din("XTF", [128, NFULL], bf16)
    XTS = din("XTS", [128, NSPAD], bf16)
    IDX16 = din("IDX16", [128, NBAT * G * 8], i16)
    EAV = din("EAV", [128, 2 * T], bf16)
    SEN = din("SEN", [128, T * BLK], bf16)
    SNE = din("SNE", [BLK, T * 128], bf16)
    W1 = {k: din(k, [128, 256], bf16) for k in ("WK1", "WV1", "WQ1", "WS1")}
    B1 = {k: din(k, [128, 256]) for k in ("BK1", "BV1", "BQ1", "BS1")}
    WE1 = din("WE1", [128, 256])
    W2 = {k: din(k, [128, 512], bf16) for k in ("WK2", "WV2", "WQ2", "WS2")}
    B2 = {k: din(k, [128, 256]) for k in ("BK2", "BV2", "BQ2", "BS2")}
    WE2 = din("WE2", [128, 256])
    G1T, B1TT = din("G1T", [128, 2]), din("B1TT", [128, 2])
    G2T, B2TT = din("G2T", [128, 2]), din("B2TT", [128, 2])
    OUTT = nc.dram_tensor("OUTT", [256, NSPAD], f32, kind="ExternalOutput").ap()

    from contextlib import ExitStack

    with tile.TileContext(nc) as tc, ExitStack() as ctx:
        cp = ctx.enter_context(tc.tile_pool(name="const", bufs=1))
        dram = ctx.enter_context(tc.tile_pool(name="dram", bufs=1, space="DRAM"))
        lhsp = ctx.enter_context(tc.tile_pool(name="lhsp", bufs=2))
        kvtp = ctx.enter_context(tc.tile_pool(name="kvtp", bufs=2))
        kvt2p = ctx.enter_context(tc.tile_pool(name="kvt2p", bufs=2))
        qsp = ctx.enter_context(tc.tile_pool(name="qsp", bufs=1))
        kvgp = ctx.enter_context(tc.tile_pool(name="kvgp", bufs=4))
        vvp = ctx.enter_context(tc.tile_pool(name="vvp", bufs=8))
        scrp = ctx.enter_context(tc.tile_pool(name="scrp", bufs=8))
        smp = ctx.enter_context(tc.tile_pool(name="smp", bufs=16))
        msop = ctx.enter_context(tc.tile_pool(name="msop", bufs=4))
        hop = ctx.enter_context(tc.tile_pool(name="hop", bufs=4))
        htp = ctx.enter_context(tc.tile_pool(name="htp", bufs=1))
        atp = ctx.enter_context(tc.tile_pool(name="atp", bufs=1))
        bigp = ctx.enter_context(tc.tile_pool(name="bigp", bufs=1))
        gpp = ctx.enter_context(tc.tile_pool(name="gpp", bufs=2, space="PSUM"))
        qgpp = ctx.enter_context(tc.tile_pool(name="qgpp", bufs=2, space="PSUM"))
        segp = ctx.enter_context(tc.tile_pool(name="segp", bufs=2, space="PSUM"))
        segdp = ctx.enter_context(tc.tile_pool(name="segdp", bufs=2, space="PSUM"))

        def pj_tile():
            return gpp.tile([128, 512], f32, name="pj", tag="gp")

        def evict(out_ap, ps_ap, bias_tile):
            if zero_bias:
                nc.any.tensor_copy(out=out_ap, in_=ps_ap)
            else:
                nc.any.tensor_tensor(out=out_ap, in0=ps_ap, in1=bias_tile,
                                     op=Alu.add)

        # ---- DRAM scratch ----
        KV1 = dram.tile([NFULL, 512], bf16, name="KV1")
        MS1 = dram.tile([2, NSPAD, 130], f32, name="MS1")
        KV2S = dram.tile([NS, 512], bf16, name="KV2S")
        KV2F = dram.tile([N, 512], bf16, name="KV2F", addr_space="Shared")
        MS2 = dram.tile([2, NSPAD, 130], f32, name="MS2")
        STI1 = dram.tile([128, 4], f32, name="STI1")
        STO1 = dram.tile([128 * R, 4], f32, name="STO1", addr_space="Shared")
        STI2 = dram.tile([128, 4], f32, name="STI2")
        STO2 = dram.tile([128 * R, 4], f32, name="STO2", addr_space="Shared")

        # ---- resident SBUF constants ----
        def load(name, ap, shape, dtype=f32):
            t = cp.tile(shape, dtype, name=name)
            nc.sync.dma_start(t[:], ap[:, :])
            return t

        xts = load("xts", XTS, [128, NSPAD], bf16)
        w1 = {k: load(k.lower(), v, [128, 256], bf16) for k, v in W1.items()}
        we1 = load("we1", WE1, [128, 256])
        # edge-phase constants loaded EARLY so layer-1 gathers can start as
        # soon as the first KV rows land (they sit on the same sync DMA queue)
        idx16s = load("idx16s", IDX16, [128, NBAT * G * 8], i16)
        eavs = load("eavs", EAV, [128, 2 * T], bf16)
        w2 = {k: load(k.lower(), v, [128, 512], bf16) for k, v in W2.items()}
        if zero_bias:
            b1 = {k: None for k in B1}
            b2 = {k: None for k in B2}
        else:
            b1 = {k: load(k.lower(), v, [128, 256]) for k, v in B1.items()}
            b2 = {k: load(k.lower(), v, [128, 256]) for k, v in B2.items()}
        we2 = load("we2", WE2, [128, 256])
        g1t = load("g1t", G1T, [128, 2])
        b1tt = load("b1tt", B1TT, [128, 2])
        g2t = load("g2t", G2T, [128, 2])
        b2tt = load("b2tt", B2TT, [128, 2])
        ident = cp.tile([128, 128], f32, name="ident")
        make_identity(nc, ident[:])

        ht = [htp.tile([128, NSPAD], f32, name=f"ht{h}") for h in range(2)]

        # ================= q projections (per 64-node block, Q stays in SBUF)
        def q_proj(lhs_chunks, wq, bq, we, lname):
            qs = qsp.tile([BLK, NB, 264], bf16, name="qs")
            nkc = len(lhs_chunks)
            for b in range(NB):
                sl = slice(b * BLK, (b + 1) * BLK)
                ps = pj_tile()
                for kc in range(nkc):
                    nc.tensor.matmul(ps[0:BLK, 0:256], lhsT=lhs_chunks[kc][:, sl],
                                     rhs=wq[:, kc * 256:(kc + 1) * 256],
                                     start=(kc == 0), stop=(kc == nkc - 1))
                if zero_bias:
                    nc.any.tensor_copy(out=qs[:, b, 0:256], in_=ps[0:BLK, 0:256])
                else:
                    nc.any.tensor_tensor(out=qs[:, b, 0:256],
                                         in0=ps[0:BLK, 0:256],
                                         in1=bq[0:BLK, :], op=Alu.add)
                scr = scrp.tile([128, 256], bf16, name="scr")
                nc.vector.tensor_mul(out=scr[0:BLK, :], in0=ps[0:BLK, 0:256],
                                     in1=we[0:BLK, 0:256])
                tsum = smp.tile([BLK, 2], f32, name="tsum")
                nc.vector.reduce_sum(
                    out=tsum[:],
                    in_=scr[0:BLK, :].rearrange("p (h c) -> p h c", c=128),
                    axis=AX.X)
                nc.any.tensor_copy(out=qs[:, b, 256:258], in_=tsum[:])
            return qs

        # q first: edge-phase compute for block b only needs qs[b], while the
        # K/V table fills behind it (src-sorted tiles gate on row prefixes).
        qs1 = q_proj([xts], w1["WQ1"], b1["BQ1"], we1, "qs1")

        # ================= layer-1 K/V projections =================
        # K/V for ALL nodes, replicated on every core.
        nt = 0
        lhs = None
        while nt < NT1:
            if nt % 8 == 0:
                nl = min(8, NT1 - nt)
                lhs = lhsp.tile([128, 8 * 128], bf16, name="lhs")
                nc.scalar.dma_start(lhs[:, 0:nl * 128],
                                    XTF[:, nt * 128:(nt + nl) * 128])
                lbase = nt
            nj = min(4, NT1 - nt)
            kvt = kvtp.tile([128, 4 * 512], bf16, name="kvt")
            for j in range(nj):
                jj = nt - lbase + j
                ps = pj_tile()
                for Wn, off in (("WK1", 0), ("WV1", 256)):
                    nc.tensor.matmul(ps[:, off:off + 256],
                                     lhsT=lhs[:, jj * 128:(jj + 1) * 128],
                                     rhs=w1[Wn][:], start=True, stop=True)
                if zero_bias:
                    # split the eviction across Scalar and Vector so the
                    # PSUM->SBUF copies run in parallel with the matmuls
                    nc.scalar.copy(out=kvt[:, j * 512:j * 512 + 256],
                                   in_=ps[:, 0:256])
                    nc.vector.tensor_copy(
                        out=kvt[:, j * 512 + 256:(j + 1) * 512],
                        in_=ps[:, 256:512])
                else:
                    for Bn, off in (("BK1", 0), ("BV1", 256)):
                        evict(kvt[:, j * 512 + off:j * 512 + off + 256],
                              ps[:, off:off + 256], b1[Bn][:])
            nc.sync.dma_start(
                KV1[nt * 128:(nt + nj) * 128, :].rearrange("(j p) c -> p j c", p=128),
                kvt[:, 0:nj * 512].rearrange("p (j c) -> p j c", c=512))
            nt += nj

        # selection matrices: needed only once edge compute starts, so load
        # them behind the projection operand traffic on the sync DMA queue
        sens = cp.tile([128, T * BLK], bf16, name="sens")
        nc.scalar.dma_start(sens[:], SEN[:, :])
        snes = cp.tile([BLK, T * 128], bf16, name="snes")
        nc.scalar.dma_start(snes[:], SNE[:, :])

        # ================= edge phase =================
        def edge_phase(KVt, idxs, qs, MSt, tile_rdep=None, round_to=512,
                       on_block=None):
            kv_rows = KVt.shape[0]
            # tile -> (block, is_first_in_block, is_last_in_block)
            tinfo = []
            for b in range(NB):
                nb = int(n_b[b])
                for i in range(nb):
                    tinfo.append((b, i == 0, i == nb - 1))
            assert len(tinfo) == T
            state = {"msv": None}

            def phase1(t0, g_n):
                """Gather + alpha for a batch -> (kvg, exfs)."""
                if tile_rdep is None:
                    rows = kv_rows
                else:
                    rmax = int(max(tile_rdep[t0:t0 + g_n]))
                    rows = min(kv_rows,
                               ((rmax + round_to - 1) // round_to) * round_to)
                k = t0 // G
                kvg = kvgp.tile([128, G, 512], bf16, name="kvg")
                nc.gpsimd.dma_gather(
                    out_ap=kvg[:, 0:g_n, :], in_ap=KVt[0:rows, :],
                    idxs_ap=idxs[:, k * (G * 8):k * (G * 8) + g_n * 8],
                    num_idxs=g_n * 128, num_idxs_reg=g_n * 128,
                    elem_size=512, queue_num=k % NQ)
                eats = smp.tile([128, G, 2], f32, name="eats")
                for g in range(g_n):
                    t = t0 + g
                    b = tinfo[t][0]
                    qgps = qgpp.tile([128, 258], f32, name="qgps", tag="qg")
                    nc.tensor.matmul(out=qgps[:],
                                     lhsT=snes[:, t * 128:(t + 1) * 128],
                                     rhs=qs[:, b, 0:258], start=True, stop=True)
                    # scr layout [p, h, 129]: cols 0:128 = qg*kg, col 128 =
                    # t[dst]*ea -> one reduce yields eat directly.
                    scr = scrp.tile([128, 258], bf16, name="scr")
                    scr3 = scr[:].rearrange("p (h c) -> p h c", c=129)
                    nc.vector.tensor_mul(
                        out=scr3[:, :, 0:128],
                        in0=qgps[:, 0:256].rearrange("p (h c) -> p h c", c=128),
                        in1=kvg[:, g, 0:256].rearrange("p (h c) -> p h c", c=128))
                    nc.vector.tensor_mul(
                        out=scr3[:, :, 128:129],
                        in0=qgps[:, 256:258].rearrange("p (h c) -> p h c", c=1),
                        in1=eavs[:, 2 * t + 1:2 * t + 2].to_broadcast([128, 2, 1]))
                    nc.vector.reduce_sum(out=eats[:, g, :], in_=scr3, axis=AX.X)
                exfs = smp.tile([128, G, 2], f32, name="exfs")
                nc.scalar.activation(out=exfs[:, 0:g_n, :],
                                     in_=eats[:, 0:g_n, :], func=Act.Exp)
                return kvg, exfs

            def phase2(t0, g_n, kvg, exfs):
                """exp-scaled selection + segment-sum matmuls for a batch.

                sens_ex[e, (h,n)] = sens[e, n] * exp(alpha)[e, h]; the V
                aggregation, softmax denominators and ea-sums then come out
                of two matmuls with rhs = raw gathered V and [1|ea] columns:
                  msv[(h,n), 0:256]   += sens_ex^T @ v      (cols h*128.. valid)
                  msv[(h,n), 256:258] += sens_ex^T @ [1|ea] (den, eas)
                """
                for g in range(g_n):
                    t = t0 + g
                    b, first, last = tinfo[t]
                    sex = vvp.tile([128, 128], bf16, name="sex")
                    nc.vector.tensor_mul(
                        out=sex[:].rearrange("p (h n) -> p h n", n=BLK),
                        in0=sens[:, t * BLK:(t + 1) * BLK].unsqueeze(1)
                            .to_broadcast([128, 2, BLK]),
                        in1=exfs[:, g, :].unsqueeze(2).to_broadcast([128, 2, BLK]))
                    if first:
                        # separate PSUM tiles: interleaved start/stop groups
                        # sharing one PSUM tile accumulate incorrectly on HW
                        state["msv"] = segp.tile([128, 256], f32, name="msv")
                        state["msd"] = segdp.tile([128, 2], f32, name="msd")
                    nc.tensor.matmul(out=state["msv"][:],
                                     lhsT=sex[:], rhs=kvg[:, g, 256:512],
                                     start=first, stop=last)
                    nc.tensor.matmul(out=state["msd"][:],
                                     lhsT=sex[:], rhs=eavs[:, 2 * t:2 * t + 2],
                                     start=first, stop=last)
                    if last:
                        for h in range(2):
                            moh = msop.tile([BLK, 130], f32, name="moh")
                            nc.any.tensor_copy(
                                out=moh[:, 0:128],
                                in_=state["msv"][h * 64:(h + 1) * 64,
                                                 h * 128:(h + 1) * 128])
                            nc.any.tensor_copy(
                                out=moh[:, 128:130],
                                in_=state["msd"][h * 64:(h + 1) * 64, :])
                            nc.sync.dma_start(
                                MSt[h, b * BLK:(b + 1) * BLK, :], moh[:])
                        if on_block is not None:
                            on_block(b)

            # software-pipelined emission: phase1(k+1) is enqueued before
            # phase2(k) so the vector queue never stalls on exp(k)
            batches = [(t0, min(G, T - t0)) for t0 in range(0, T, G)]
            prev = None
            for t0, g_n in batches:
                cur = (t0, g_n) + phase1(t0, g_n)
                if prev is not None:
                    phase2(*prev)
                prev = cur
            phase2(*prev)

        def bail():
            z = bigp.tile([128, NSPAD], f32, name="sq")
            nc.vector.memset(z[:], 0.0)
            for half in range(2):
                nc.sync.dma_start(OUTT[half * 128:(half + 1) * 128, :], z[:])

        # ================= node phase =================
        def node_tile(mt, MSt, lhs_chunks, ws, bs, we):
                nkc = len(lhs_chunks)
                sl = slice(mt * 128, (mt + 1) * 128)
                # per-head message tiles for this 128-node slice (2 blocks):
                # cols 0:128 = msg_h, col 128 = den_h, col 129 = eas_h
                msh = []
                for h in range(2):
                    m = msop.tile([128, 130], f32, name=f"msi{h}")
                    nc.sync.dma_start(m[:], MSt[h, sl, :])
                    msh.append(m)
                ps = pj_tile()
                for kc in range(nkc):
                    nc.tensor.matmul(ps[:, 0:256], lhsT=lhs_chunks[kc][:, sl],
                                     rhs=ws[:, kc * 256:(kc + 1) * 256],
                                     start=(kc == 0), stop=(kc == nkc - 1))
                for h in range(2):
                    m = msh[h]
                    den = smp.tile([128, 1], f32, name="den")
                    nc.vector.tensor_scalar_add(out=den[:], in0=m[:, 128:129],
                                                scalar1=1e-16)
                    rec = smp.tile([128, 1], f32, name="rec")
                    nc.vector.reciprocal(out=rec[:], in_=den[:])
                    tmp = hop.tile([128, 128], f32, name="tmpn")
                    # tmp = we_h * eas_h + msg_h
                    nc.vector.scalar_tensor_tensor(
                        out=tmp[:], in0=we[:, h * 128:(h + 1) * 128],
                        scalar=m[:, 129:130], in1=m[:, 0:128],
                        op0=Alu.mult, op1=Alu.add)
                    ho = hop.tile([128, 128], f32, name="ho")
                    # ho = tmp / den + skip-projection
                    nc.vector.scalar_tensor_tensor(
                        out=ho[:], in0=tmp[:], scalar=rec[:],
                        in1=ps[:, h * 128:(h + 1) * 128],
                        op0=Alu.mult, op1=Alu.add)
                    if not zero_bias:
                        nc.vector.tensor_add(out=ho[:], in0=ho[:],
                                             in1=bs[:, h * 128:(h + 1) * 128])
                    tp = gpp.tile([128, 512], f32, name="pj", tag="gp")
                    nc.tensor.transpose(out=tp[:, 0:128], in_=ho[:],
                                        identity=ident[:])
                    nc.vector.tensor_copy(out=ht[h][:, sl], in_=tp[:, 0:128])

        def node_finale(gt, bt, at_names, STI, STO, at_dt):
            stt = smp.tile([128, 4], f32, name="stt")
            for half in range(2):
                nc.vector.reduce_sum(out=stt[:, half:half + 1],
                                     in_=ht[half][:, 0:NS], axis=AX.X)
                sq = bigp.tile([128, NSPAD], f32, name="sq")
                nc.scalar.activation(out=sq[:, 0:NS], in_=ht[half][:, 0:NS],
                                     func=Act.Square,
                                     accum_out=stt[:, 2 + half:3 + half])
            nc.sync.dma_start(STI[:, :], stt[:])
            nc.gpsimd.collective_compute(
                "AllGather", Alu.bypass, replica_groups=[list(range(R))],
                ins=[STI[:].opt()], outs=[STO[:].opt()])
            gsta = smp.tile([128, 4, R], f32, name="gsta")
            nc.sync.dma_start(
                gsta[:], STO[:].rearrange("(r p) k -> p k r", p=128))
            gst = smp.tile([128, 4], f32, name="gst")
            nc.vector.reduce_sum(out=gst[:], in_=gsta[:], axis=AX.X)
            mean = smp.tile([128, 2], f32, name="mean")
            nc.vector.tensor_scalar_mul(out=mean[:], in0=gst[:, 0:2],
                                        scalar1=1.0 / N)
            var = smp.tile([128, 2], f32, name="var")
            nc.vector.tensor_scalar_mul(out=var[:], in0=gst[:, 2:4],
                                        scalar1=1.0 / N)
            m2 = smp.tile([128, 2], f32, name="m2")
            nc.vector.tensor_mul(out=m2[:], in0=mean[:], in1=mean[:])
            nc.vector.tensor_sub(out=var[:], in0=var[:], in1=m2[:])
            nc.vector.tensor_scalar_add(out=var[:], in0=var[:], scalar1=EPS)
            sd = smp.tile([128, 2], f32, name="sd")
            nc.scalar.activation(out=sd[:], in_=var[:], func=Act.Sqrt)
            rsd = smp.tile([128, 2], f32, name="rsd")
            nc.vector.reciprocal(out=rsd[:], in_=sd[:])
            sc2 = smp.tile([128, 2], f32, name="sc2")
            nc.vector.tensor_mul(out=sc2[:], in0=gt[:], in1=rsd[:])
            sh2 = smp.tile([128, 2], f32, name="sh2")
            nc.vector.tensor_mul(out=sh2[:], in0=mean[:], in1=sc2[:])
            nc.vector.tensor_sub(out=sh2[:], in0=bt[:], in1=sh2[:])
            at = [atp.tile([128, NSPAD], at_dt, name=nm) for nm in at_names]
            for half in range(2):
                nc.scalar.activation(out=at[half][:], in_=ht[half][:],
                                     func=Act.Relu,
                                     scale=sc2[:, half:half + 1],
                                     bias=sh2[:, half:half + 1])
            return at

        if stage >= 2:
            def cb1(b):
                # emit node-tile work lagged two blocks behind the eviction
                # so its loads never wait at the head of the sync queue
                if b % 2 == 1 and b >= 3 and stage >= 3:
                    node_tile((b - 3) // 2, MS1, [xts], w1["WS1"], b1["BS1"],
                              we1)
            edge_phase(KV1, idx16s, qs1, MS1, tile_rdep=rdep, on_block=cb1)
        if stage >= 3:
            node_tile(NTS - 2, MS1, [xts], w1["WS1"], b1["BS1"], we1)
            node_tile(NTS - 1, MS1, [xts], w1["WS1"], b1["BS1"], we1)
            at1 = node_finale(g1t, b1tt, ["at10", "at11"], STI1, STO1, bf16)

        # ===== layer-2 K/V projections (data-parallel + chunked AllGather) ===
        if stage >= 4:
          for mt in range(NTS):
            sl = slice(mt * 128, (mt + 1) * 128)
            kvt = kvt2p.tile([128, 512], bf16, name="kvt2")
            ps = pj_tile()
            for Wn, off in (("WK2", 0), ("WV2", 256)):
                for kc in range(2):
                    nc.tensor.matmul(ps[:, off:off + 256], lhsT=at1[kc][:, sl],
                                     rhs=w2[Wn][:, kc * 256:(kc + 1) * 256],
                                     start=(kc == 0), stop=(kc == 1))
            if zero_bias:
                nc.any.tensor_copy(out=kvt[:], in_=ps[:])
            else:
                for Bn, off in (("BK2", 0), ("BV2", 256)):
                    evict(kvt[:, off:off + 256], ps[:, off:off + 256],
                          b2[Bn][:])
            if mt < NTS - 1:
                nc.sync.dma_start(KV2S[sl, :], kvt[:])
            else:
                nc.sync.dma_start(KV2S[mt * 128:NS, :],
                                  kvt[:NS - mt * 128, :])
          nc.gpsimd.collective_compute(
              "AllGather", Alu.bypass, replica_groups=[list(range(R))],
              ins=[KV2S[:].opt()], outs=[KV2F[:].opt()])
          qs2 = q_proj(at1, w2["WQ2"], b2["BQ2"], we2, "qs2")

        if stage >= 5:
            def cb2(b):
                if b % 2 == 1 and b >= 3 and stage >= 99:
                    node_tile((b - 3) // 2, MS2, at1, w2["WS2"], b2["BS2"],
                              we2)
            edge_phase(KV2F, idx16s, qs2, MS2, on_block=cb2)

        if stage >= 99:
            node_tile(NTS - 2, MS2, at1, w2["WS2"], b2["BS2"], we2)
            node_tile(NTS - 1, MS2, at1, w2["WS2"], b2["BS2"], we2)
            at2 = node_finale(g2t, b2tt, ["at20", "at21"], STI2, STO2, f32)
            for half in range(2):
                nc.sync.dma_start(OUTT[half * 128:(half + 1) * 128, :],
                                  at2[half][:])
        else:
            bail()

    nc.compile()
    return nc


def _pack_inputs(inputs, per_core):
    import ml_dtypes
    bfh = ml_dtypes.bfloat16
    x = np.asarray(inputs["x"], np.float32)
    # XTF columns in g2row order so the KV1 table (written sequentially by
    # the projection loop) lands in the same row space the layer-2
    # chunked-AllGather table uses; src indices are g2row for both layers
    xT = np.zeros((128, NFULL), np.float32)
    xT[:, :N] = x.T
    common = {"XTF": np.ascontiguousarray(xT).astype(bfh)}

    def bc(v):
        return np.ascontiguousarray(np.broadcast_to(
            np.asarray(v, np.float32).reshape(1, -1), (128, v.shape[-1])))

    def chunks2(w):
        w = np.asarray(w, np.float32)
        return np.ascontiguousarray(np.concatenate([w[0:128], w[128:256]], axis=1))

    for L, nm in ((1, "1"), (2, "2")):
        for key, wn in (("Wk", "WK"), ("Wv", "WV"), ("Wq", "WQ"), ("Ws", "WS")):
            w = np.asarray(inputs[key + nm], np.float32)
            if wn == "WQ":
                w = w * np.float32(INV)
            if L == 1:
                common[wn + nm] = np.ascontiguousarray(w).astype(bfh)
            else:
                common[wn + nm] = chunks2(w).astype(bfh)
        for key, bn in (("bk", "BK"), ("bv", "BV"), ("bq", "BQ"), ("bs", "BS")):
            b = np.asarray(inputs[key + nm], np.float32)
            if bn == "BQ":
                b = b * np.float32(INV)
            common[bn + nm] = bc(b)
        common["WE" + nm] = bc(np.asarray(inputs["We" + nm], np.float32).reshape(-1))
        common["G" + nm + "T"] = np.ascontiguousarray(
            np.asarray(inputs["g" + nm], np.float32).reshape(2, 128).T)
        common["B" + nm + "TT"] = np.ascontiguousarray(
            np.asarray(inputs["b" + nm], np.float32).reshape(2, 128).T)

    in_maps = []
    for r in range(R):
        m = dict(common)
        xs = np.zeros((128, NSPAD), np.float32)
        xs[:, :NS] = x[NS * r:NS * (r + 1)].T
        m["XTS"] = np.ascontiguousarray(xs).astype(bfh)
        m.update(per_core[r])
        in_maps.append(m)
    return in_maps


def kernel(**inputs):
    from concourse import bass_utils

    edge_index = np.asarray(inputs["edge_index"])
    edge_attr = np.asarray(inputs["edge_attr"], np.float32)
    zb = all(not np.any(np.asarray(inputs[k]))
             for k in ("bq1", "bk1", "bv1", "bs1", "bq2", "bk2", "bv2", "bs2"))
    key = (hash(edge_index.tobytes()), zb)
    if key not in _CACHE:
        per_core, n_b, T, rdep = _prepare(edge_index, edge_attr)
        nc = _build(T, n_b, rdep=rdep, zero_bias=zb)
        _CACHE[key] = (nc, per_core)
    nc, per_core = _CACHE[key]

    in_maps = _pack_inputs(inputs, per_core)
    import os
    trace = bool(int(os.environ.get("KBENCH_TRACE", "0")))
    res = bass_utils.run_bass_kernel_spmd(
        nc, in_maps, core_ids=list(range(R)), trace=trace)
    kernel.last_result = res
    out = np.concatenate(
        [res.results[r]["OUTT"][:, :NS].T for r in range(R)], axis=0)
    return np.ascontiguousarray(out)

